# revision 7
# baseline (speedup 1.0000x reference)
"""Causal self-attention Trainium2 kernel (v2: all-bf16).

B=2, T=2048, C=1024, H=16, D=64, 8 NeuronCores.
Sharding: core i handles batch b=i//4 and heads [4*(i%4), 4*(i%4)+4).
Host converts inputs to bf16 (and pre-transposes x), slices weights per
core, and sums the 4 per-batch partial output projections at the end.

All matmuls run in bf16 (1 cyc/row, 2-byte weight loads). Scores are
computed transposed (S^T[j,i]) so softmax exp/mask are free-dim ops and
P^T feeds the attention*V matmul as the moving operand. A ones column
per v chunk yields the softmax denominator for free. exp() runs on ACT
in 1-2 PSUM-bank tiles to amortize the per-instruction access latency;
ACT only ever runs Exp (reciprocal lives on DVE) so its function table
never reloads. PSUM->SBUF staging copies run on Pool to keep DVE under
its budget.
"""

import numpy as np
import ml_dtypes
from contextlib import ExitStack

import concourse.bacc as bacc
import concourse.mybir as mybir
import concourse.tile as tile
from concourse.bass_utils import run_bass_kernel_spmd

B, T, C = 2, 2048, 1024
N_HEAD = 16
D = C // N_HEAD  # 64
N_CORES = 8
HPC = 4  # heads per core
TB = T // 512  # 4 i-blocks of 512
NJ = T // 128  # 16 j-chunks of 128

F32 = mybir.dt.float32
F32R = mybir.dt.float32r
BF16 = mybir.dt.bfloat16
NPBF = ml_dtypes.bfloat16

_compiled = None


def _build_trim():
    """trim01[j, q*128 + ii] = 1.0 iff j <= ii, for q in {0,1}; [128, 256].

    Mask for the first 128 written columns of each diagonal S^T block.
    """
    j = np.arange(128)[:, None]
    ii = np.arange(128)[None, :]
    tri = (j <= ii).astype(NPBF)
    return np.concatenate([tri, tri], axis=1)


def _build_nc():
    nc = bacc.Bacc("TRN2", target_bir_lowering=False, debug=False,
                   num_devices=N_CORES)

    xt_t = nc.dram_tensor("xt", [128, 8 * T], BF16, kind="ExternalInput")
    wqk_t = nc.dram_tensor("wqk", [128, 8 * 512], BF16, kind="ExternalInput")
    wv_t = nc.dram_tensor("wv", [128, 8 * 256], BF16, kind="ExternalInput")
    wp_t = nc.dram_tensor("wp", [128, 2 * C], BF16, kind="ExternalInput")
    trim_t = nc.dram_tensor("trim", [128, 256], BF16, kind="ExternalInput")
    sel_t = nc.dram_tensor("sel", [4, 4 * 64], F32R, kind="ExternalInput")
    out_t = nc.dram_tensor("out", [T, C], BF16, kind="ExternalOutput")

    Exp = mybir.ActivationFunctionType.Exp

    with tile.TileContext(nc) as tc, ExitStack() as ctx:
        sb = ctx.enter_context(tc.tile_pool(name="sb", bufs=1))
        # PSUM: ps2 2x[128,1024] (4 banks) + psy 2 (2) + psb 2 (2) = 8
        ps2 = ctx.enter_context(
            tc.tile_pool(name="ps2", bufs=2, space="PSUM"))
        psy = ctx.enter_context(
            tc.tile_pool(name="psy", bufs=2, space="PSUM"))
        psb = ctx.enter_context(
            tc.tile_pool(name="psb", bufs=2, space="PSUM"))

        # ---- persistent SBUF ----
        trim_s = sb.tile([128, 256], BF16, tag="trim")
        sel_s = sb.tile([4, 4 * 64], F32R, tag="sel")
        wp_s = [sb.tile([128, C], BF16, tag=f"wp{p}", name=f"wp{p}")
                for p in range(2)]
        # qT/kT pair tiles: rows 0-63 = head 2p, rows 64-127 = head 2p+1
        qT = [sb.tile([128, T], BF16, tag=f"qT{p}", name=f"qT{p}")
              for p in range(2)]
        kT = [sb.tile([128, T], BF16, tag=f"kT{p}", name=f"kT{p}")
              for p in range(2)]
        # v per head: [128 t-part, 65*NJ]; chunk jc: cols 0-63 v, col 64 = 1
        v_s = [sb.tile([128, 65 * NJ], BF16, tag=f"v{h}", name=f"v{h}")
               for h in range(HPC)]
        yT = [sb.tile([128, T], BF16, tag=f"yT{p}", name=f"yT{p}")
              for p in range(2)]
        ytmp = sb.tile([64, 8 * 512], BF16, tag="ytmp")

        # ones columns of v (col 64 of each 65-wide chunk)
        for h in range(HPC):
            nc.vector.memset(
                v_s[h][:].rearrange("p (c w) -> p c w", w=65)[:, :, 64:65],
                1.0)

        # ---- phase A: qkv projections ----
        with tc.tile_pool(name="pa", bufs=1) as pa:
            wqk_s = [pa.tile([128, 512], BF16, tag=f"wqk{kc}",
                             name=f"wqk{kc}") for kc in range(8)]
            for kc in range(8):
                nc.sync.dma_start(wqk_s[kc][:],
                                  wqk_t.ap()[:, 512 * kc:512 * (kc + 1)])
            xt_s = {}
            for nb in range(TB):
                for kc in range(8):
                    t = pa.tile([128, 512], BF16, tag=f"xt{kc}_{nb}",
                                name=f"xt{kc}_{nb}")
                    nc.sync.dma_start(
                        t[:], xt_t.ap()[:, T * kc + 512 * nb:
                                        T * kc + 512 * (nb + 1)])
                    xt_s[(kc, nb)] = t
                if nb == 0:
                    wv_s = [pa.tile([128, 256], BF16, tag=f"wv{kc}",
                                    name=f"wv{kc}") for kc in range(8)]
                    for kc in range(8):
                        nc.sync.dma_start(
                            wv_s[kc][:],
                            wv_t.ap()[:, 256 * kc:256 * (kc + 1)])
                    with tc.tile_wait_until(0.012):
                        nc.scalar.dma_start(trim_s[:], trim_t.ap()[:])
                        nc.scalar.dma_start(sel_s[:], sel_t.ap()[:])
                        for p in range(2):
                            nc.scalar.dma_start(
                                wp_s[p][:],
                                wp_t.ap()[:, C * p:C * (p + 1)])
            for nb in range(TB):
                # qk: one [128,1024] psum per mc-pair; halves mc0/mc1
                for mcp, dsts in ((0, qT), (1, kT)):
                    p = ps2.tile([128, 1024], F32, tag="mm")
                    for half in range(2):
                        mc = 2 * mcp + half
                        for kc in range(8):
                            nc.tensor.matmul(
                                p[:, 512 * half:512 * (half + 1)],
                                wqk_s[kc][:, 128 * mc:128 * (mc + 1)],
                                xt_s[(kc, nb)][:],
                                start=(kc == 0), stop=(kc == 7))
                    for half in range(2):
                        nc.vector.tensor_copy(
                            dsts[half][:, 512 * nb:512 * (nb + 1)],
                            p[:, 512 * half:512 * (half + 1)])
                # v: quarters of one [128,1024] psum; out [128 t, 256]
                p = ps2.tile([128, 1024], F32, tag="mm")
                for tq in range(4):
                    for kc in range(8):
                        nc.tensor.matmul(
                            p[:, 256 * tq:256 * (tq + 1)],
                            xt_s[(kc, nb)][:, 128 * tq:128 * (tq + 1)],
                            wv_s[kc][:], start=(kc == 0), stop=(kc == 7))
                for tq in range(4):
                    tci = 4 * nb + tq
                    for h in range(HPC):
                        nc.vector.tensor_copy(
                            v_s[h][:, 65 * tci:65 * tci + 64],
                            p[:, 256 * tq + 64 * h:256 * tq + 64 * (h + 1)])

        with tc.tile_pool(name="pb", bufs=1) as pb:
            # ---- phase B: attention ----
            drows = [pb.tile([4, 512], F32, tag=f"dr{ib}", name=f"dr{ib}")
                     for ib in range(TB)]

            def make_normproj(ib):
                def fin():
                    recs = pb.tile([4, 512], F32R, tag="recs", bufs=2,
                                   name="recs")
                    # f32r shares f32 bits; TF32 rounding only on PE read
                    with nc.allow_low_precision(reason="f32r == f32 bits"):
                        nc.vector.reciprocal(recs[:], drows[ib][:])
                    for h in range(HPC):
                        pr, sub = h // 2, h % 2
                        prf = psb.tile([64, 512], F32, tag="bcast")
                        nc.tensor.matmul(prf[:],
                                         sel_s[:, 64 * h:64 * (h + 1)],
                                         recs[:], start=True, stop=True)
                        if sub == 0:
                            nc.vector.tensor_mul(
                                yT[pr][0:64, 512 * ib:512 * (ib + 1)],
                                yT[pr][0:64, 512 * ib:512 * (ib + 1)],
                                prf[:])
                        else:
                            oidx = 4 * pr + ib
                            blk = ytmp[:, 512 * oidx:512 * (oidx + 1)]
                            nc.vector.tensor_mul(blk, blk, prf[:])
                            nc.scalar.dma_start(
                                yT[pr][64:128, 512 * ib:512 * (ib + 1)],
                                blk)
                    # output projection for the 4 t-chunks of this ib
                    for tb in range(4 * ib, 4 * ib + 4):
                        p = ps2.tile([128, 1024], F32, tag="mm")
                        for n in range(2):
                            for pp in range(2):
                                nc.tensor.matmul(
                                    p[:, 512 * n:512 * (n + 1)],
                                    yT[pp][:, 128 * tb:128 * (tb + 1)],
                                    wp_s[pp][:, 512 * n:512 * (n + 1)],
                                    start=(pp == 0), stop=(pp == 1))
                        o = pb.tile([128, 1024], BF16, tag="o", bufs=2,
                                    name="o")
                        nc.vector.tensor_copy(o[:], p[:])
                        nc.sync.dma_start(
                            out_t.ap()[128 * tb:128 * (tb + 1), :], o[:])
                return fin

            pending = None
            for h in range(HPC):
                pr, sub = h // 2, h % 2
                qTr = qT[pr][64 * sub:64 * (sub + 1), :]
                kTr = kT[pr][64 * sub:64 * (sub + 1), :]
                for ib in range(TB):
                    jhi = 4 * ib + 3
                    i0 = 512 * ib
                    py = psy.tile([65, 512], F32, tag="avy")
                    avq = []

                    def emit_av(ent, py=py, h=h, jhi=jhi):
                        # list of (jc, pt_ap, out_off) AV matmuls
                        for jc, mov, off in ent:
                            nc.tensor.matmul(
                                py[:, off:512],
                                v_s[h][:, 65 * jc:65 * jc + 65],
                                mov, start=(jc == 0), stop=(jc == jhi))

                    # units: full pairs, then two diagonal pair-tiles
                    units = [("full", jc) for jc in range(0, 4 * ib, 2)]
                    units.append(("diagA", 4 * ib))
                    units.append(("diagB", 4 * ib + 2))
                    for ui, (kind, jc0) in enumerate(units):
                        p = ps2.tile([128, 1024], F32, tag="mm")
                        pt = pb.tile([128, 1024], BF16, tag="P", bufs=4,
                                     name="pt")
                        if kind == "full":
                            for half in range(2):
                                nc.tensor.matmul(
                                    p[:, 512 * half:512 * (half + 1)],
                                    kTr[:, 128 * (jc0 + half):
                                        128 * (jc0 + half + 1)],
                                    qTr[:, i0:i0 + 512],
                                    start=True, stop=True)
                            nc.scalar.activation(pt[:], p[:], Exp,
                                                 scale=0.125)
                            av = [(jc0, pt[:, 0:512], 0),
                                  (jc0 + 1, pt[:, 512:1024], 0)]
                        elif kind == "diagA":
                            # r0: w=512 at [0:512]; r1: w=384 at [512:896]
                            nc.tensor.matmul(
                                p[:, 0:512],
                                kTr[:, 128 * jc0:128 * (jc0 + 1)],
                                qTr[:, i0:i0 + 512],
                                start=True, stop=True)
                            nc.tensor.matmul(
                                p[:, 512:896],
                                kTr[:, 128 * (jc0 + 1):128 * (jc0 + 2)],
                                qTr[:, i0 + 128:i0 + 512],
                                start=True, stop=True)
                            nc.scalar.activation(pt[:, 0:896], p[:, 0:896],
                                                 Exp, scale=0.125)
                            nc.gpsimd.tensor_mul(
                                pt[:].rearrange(
                                    "p (u w) -> p u w", w=512)[:, :, 0:128],
                                pt[:].rearrange(
                                    "p (u w) -> p u w", w=512)[:, :, 0:128],
                                trim_s[:].rearrange(
                                    "p (u w) -> p u w", w=128))
                            av = [(jc0, pt[:, 0:512], 0),
                                  (jc0 + 1, pt[:, 512:896], 128)]
                        else:  # diagB
                            # r2: w=256 at [0:256]; r3: w=128 at [256:384]
                            nc.tensor.matmul(
                                p[:, 0:256],
                                kTr[:, 128 * jc0:128 * (jc0 + 1)],
                                qTr[:, i0 + 256:i0 + 512],
                                start=True, stop=True)
                            nc.tensor.matmul(
                                p[:, 256:384],
                                kTr[:, 128 * (jc0 + 1):128 * (jc0 + 2)],
                                qTr[:, i0 + 384:i0 + 512],
                                start=True, stop=True)
                            nc.scalar.activation(pt[:, 0:384], p[:, 0:384],
                                                 Exp, scale=0.125)
                            nc.gpsimd.tensor_mul(
                                pt[:, 0:512].rearrange(
                                    "p (u w) -> p u w", w=256)[:, :, 0:128],
                                pt[:, 0:512].rearrange(
                                    "p (u w) -> p u w", w=256)[:, :, 0:128],
                                trim_s[:].rearrange(
                                    "p (u w) -> p u w", w=128))
                            av = [(jc0, pt[:, 0:256], 256),
                                  (jc0 + 1, pt[:, 256:384], 384)]
                        avq.append(av)
                        if ui == 2 and pending is not None:
                            pending()
                            pending = None
                        if len(avq) > 2:
                            emit_av(avq.pop(0))
                    while avq:
                        emit_av(avq.pop(0))
                    # denominator row + unnormalized y staging
                    # (GPSIMD/DMA cannot read PSUM -> DVE)
                    dtmp = pb.tile([1, 512], F32, tag="dtmp", bufs=2)
                    nc.vector.tensor_copy(dtmp[:], py[64:65, :])
                    nc.scalar.dma_start(drows[ib][h:h + 1, :], dtmp[:])
                    if sub == 0:
                        nc.vector.tensor_copy(
                            yT[pr][0:64, i0:i0 + 512], py[0:64, :])
                    else:
                        oidx = 4 * pr + ib
                        nc.vector.tensor_copy(
                            ytmp[:, 512 * oidx:512 * (oidx + 1)],
                            py[0:64, :])
                    if h == HPC - 1:
                        pending = make_normproj(ib)
            pending()

    nc.compile()
    return nc


def _get_compiled():
    global _compiled
    if _compiled is None:
        _compiled = _build_nc()
    return _compiled


def _in_maps(x, w_qkv, w_proj):
    x = np.asarray(x, dtype=np.float32)
    w_qkv = np.asarray(w_qkv, dtype=np.float32)
    w_proj = np.asarray(w_proj, dtype=np.float32)
    trim = _build_trim()
    sel = np.zeros((4, 4 * 64), dtype=np.float32)
    for b in range(4):
        sel[b, 64 * b:64 * (b + 1)] = 1.0
    maps = []
    for core in range(N_CORES):
        b = core // 4
        h0 = 4 * (core % 4)
        heads = range(h0, h0 + HPC)
        # xt[p, kc*T + t] = x[b, t, 128*kc + p]
        xt = np.ascontiguousarray(
            x[b].T.astype(NPBF).reshape(8, 128, T).transpose(1, 0, 2)
            .reshape(128, 8 * T))
        # wqk[p, kc*512 + m]: m<256 q cols (heads x 64), else k cols
        wq = np.concatenate(
            [w_qkv[:, 64 * h:64 * (h + 1)] for h in heads], axis=1)
        wk = np.concatenate(
            [w_qkv[:, C + 64 * h:C + 64 * (h + 1)] for h in heads], axis=1)
        wqk = np.concatenate([wq, wk], axis=1).astype(NPBF)  # [C, 512]
        wqk = np.ascontiguousarray(
            wqk.reshape(8, 128, 512).transpose(1, 0, 2).reshape(128, -1))
        wv = np.concatenate(
            [w_qkv[:, 2 * C + 64 * h:2 * C + 64 * (h + 1)] for h in heads],
            axis=1).astype(NPBF)  # [C, 256]
        wv = np.ascontiguousarray(
            wv.reshape(8, 128, 256).transpose(1, 0, 2).reshape(128, -1))
        wp = np.concatenate(
            [w_proj[64 * h:64 * (h + 1), :] for h in heads],
            axis=0).astype(NPBF)  # [256, C]
        wp = np.ascontiguousarray(
            wp.reshape(2, 128, C).transpose(1, 0, 2).reshape(128, 2 * C))
        maps.append({
            "xt": xt,
            "wqk": wqk,
            "wv": wv,
            "wp": wp,
            "trim": trim,
            "sel": sel,
        })
    return maps


def _combine(results, b_proj):
    out = np.zeros((B, T, C), dtype=np.float32)
    for core in range(N_CORES):
        out[core // 4] += np.asarray(results[core]["out"],
                                     dtype=np.float32)
    out += np.asarray(b_proj, dtype=np.float32)[None, None, :]
    return out


def kernel(x, w_qkv, w_proj, b_proj):
    nc = _get_compiled()
    res = run_bass_kernel_spmd(nc, _in_maps(x, w_qkv, w_proj),
                               core_ids=list(range(N_CORES)))
    return _combine(res.results, b_proj)


def kernel_traced(x, w_qkv, w_proj, b_proj):
    """Like kernel() but with NTFF tracing; returns (out, results)."""
    nc = _get_compiled()
    res = run_bass_kernel_spmd(nc, _in_maps(x, w_qkv, w_proj),
                               core_ids=list(range(N_CORES)), trace=True)
    return _combine(res.results, b_proj), res


# revision 9
# speedup vs baseline: 1.0353x; 1.0353x over previous
"""Causal self-attention Trainium2 kernel (v3: bf16, software-pipelined).

B=2, T=2048, C=1024, H=16, D=64, 8 NeuronCores.
Sharding: core i handles batch b=i//4 and heads [4*(i%4), 4*(i%4)+4).
Host converts inputs to bf16 (and pre-transposes x), slices weights per
core, and sums the 4 per-batch partial output projections at the end.

All matmuls run in bf16. Scores are computed transposed (S^T[j,i]) so
softmax exp/mask are free-dim ops and P^T feeds the attention*V matmul
as the moving operand; a ones column per v chunk yields the softmax
denominator for free.

The attention stream alone is ACT-paced: exp of N score elements costs
N/1.2GHz while its S+AV matmuls cost 2N/2.4GHz — identical — so the PE
idles by the ACT per-instruction overhead every block, and those
micro-gaps keep the HAM activity throttle in its K=4/8 (half-clock)
state. v3 exploits causality — attention for i-block ib only reads
q/k/v t-blocks <= ib — to interleave the NEXT t-block's qkv projection
matmuls (which have no ACT dependence) into the attention stream as PE
filler, keeping the tensor engine dense and the throttle released.
Each filler chunk is self-contained (PSUM alloc + matmuls + drain cast)
so pool buffer lifetimes never interleave.
"""

import numpy as np
import ml_dtypes
from collections import deque
from contextlib import ExitStack

import concourse.bacc as bacc
import concourse.mybir as mybir
import concourse.tile as tile
from concourse.bass_utils import run_bass_kernel_spmd

B, T, C = 2, 2048, 1024
N_HEAD = 16
D = C // N_HEAD  # 64
N_CORES = 8
HPC = 4  # heads per core
TB = T // 512  # 4 i-blocks of 512
NJ = T // 128  # 16 j-chunks of 128

F32 = mybir.dt.float32
F32R = mybir.dt.float32r
BF16 = mybir.dt.bfloat16
NPBF = ml_dtypes.bfloat16

_compiled = None


def _build_trim():
    """trim01[j, ii] = 1.0 iff j <= ii; [128, 128] bf16."""
    j = np.arange(128)[:, None]
    ii = np.arange(128)[None, :]
    return (j <= ii).astype(NPBF)


def _build_nc():
    nc = bacc.Bacc("TRN2", target_bir_lowering=False, debug=False,
                   num_devices=N_CORES)

    xt_t = nc.dram_tensor("xt", [128, 8 * T], BF16, kind="ExternalInput")
    wqk_t = nc.dram_tensor("wqk", [128, 8 * 512], BF16, kind="ExternalInput")
    wv_t = nc.dram_tensor("wv", [128, 8 * 256], BF16, kind="ExternalInput")
    wp_t = nc.dram_tensor("wp", [128, 2 * C], BF16, kind="ExternalInput")
    trim_t = nc.dram_tensor("trim", [128, 128], BF16, kind="ExternalInput")
    sel_t = nc.dram_tensor("sel", [4, 4 * 64], F32R, kind="ExternalInput")
    out_t = nc.dram_tensor("out", [T, C], BF16, kind="ExternalOutput")

    Exp = mybir.ActivationFunctionType.Exp

    with tile.TileContext(nc) as tc, ExitStack() as ctx:
        sb = ctx.enter_context(tc.tile_pool(name="sb", bufs=1))
        # PSUM banks: psm 4x[128,512] (4) + psy 2 (2) + psb 2 (2) = 8
        psm = ctx.enter_context(
            tc.tile_pool(name="psm", bufs=4, space="PSUM"))
        psy = ctx.enter_context(
            tc.tile_pool(name="psy", bufs=2, space="PSUM"))
        psb = ctx.enter_context(
            tc.tile_pool(name="psb", bufs=2, space="PSUM"))

        # ---- persistent SBUF ----
        trim_s = sb.tile([128, 128], BF16, tag="trim")
        sel_s = sb.tile([4, 4 * 64], F32R, tag="sel")
        wp_s = [sb.tile([128, C], BF16, tag=f"wp{p}", name=f"wp{p}")
                for p in range(2)]
        # qT/kT pair tiles: rows 0-63 = head 2p, rows 64-127 = head 2p+1
        qT = [sb.tile([128, T], BF16, tag=f"qT{p}", name=f"qT{p}")
              for p in range(2)]
        kT = [sb.tile([128, T], BF16, tag=f"kT{p}", name=f"kT{p}")
              for p in range(2)]
        # v per head: [128 t-part, 65*NJ]; chunk jc: cols 0-63 v, col 64 = 1
        v_s = [sb.tile([128, 65 * NJ], BF16, tag=f"v{h}", name=f"v{h}")
               for h in range(HPC)]
        yT = [sb.tile([128, T], BF16, tag=f"yT{p}", name=f"yT{p}")
              for p in range(2)]
        ytmp = sb.tile([64, 8 * 512], BF16, tag="ytmp")
        drows = [sb.tile([4, 512], F32, tag=f"dr{ib}", name=f"dr{ib}")
                 for ib in range(TB)]

        for h in range(HPC):
            nc.vector.memset(
                v_s[h][:].rearrange("p (c w) -> p c w", w=65)[:, :, 64:65],
                1.0)

        # ---- input DMA (sync queue drains in order) ----
        wqk_s = [sb.tile([128, 512], BF16, tag=f"wqk{kc}", name=f"wqk{kc}")
                 for kc in range(8)]
        for kc in range(8):
            nc.sync.dma_start(wqk_s[kc][:],
                              wqk_t.ap()[:, 512 * kc:512 * (kc + 1)])
        xt_s = {}
        wv_s = []
        for nb in range(TB):
            for kc in range(8):
                t = sb.tile([128, 512], BF16, tag=f"xt{kc}_{nb}",
                            name=f"xt{kc}_{nb}")
                nc.sync.dma_start(
                    t[:], xt_t.ap()[:, T * kc + 512 * nb:
                                    T * kc + 512 * (nb + 1)])
                xt_s[(kc, nb)] = t
            if nb == 0:
                wv_s = [sb.tile([128, 256], BF16, tag=f"wv{kc}",
                                name=f"wv{kc}") for kc in range(8)]
                for kc in range(8):
                    nc.sync.dma_start(
                        wv_s[kc][:], wv_t.ap()[:, 256 * kc:256 * (kc + 1)])
                with tc.tile_wait_until(0.012):
                    nc.scalar.dma_start(trim_s[:], trim_t.ap()[:])
                    nc.scalar.dma_start(sel_s[:], sel_t.ap()[:])
                    for p in range(2):
                        nc.scalar.dma_start(
                            wp_s[p][:], wp_t.ap()[:, C * p:C * (p + 1)])

        # ---- qkv projection chunks (self-contained PE filler) ----
        def qk_chunk(nb, mc):
            def run():
                p = psm.tile([128, 512], F32, tag="mm")
                for kc in range(8):
                    nc.tensor.matmul(
                        p[:], wqk_s[kc][:, 128 * mc:128 * (mc + 1)],
                        xt_s[(kc, nb)][:], start=(kc == 0), stop=(kc == 7))
                dst = qT[mc] if mc < 2 else kT[mc - 2]
                nc.vector.tensor_copy(
                    dst[:, 512 * nb:512 * (nb + 1)], p[:])
            return run

        def v_chunk(nb, tq):
            def run():
                p = psm.tile([128, 512], F32, tag="mm")
                for kc in range(8):
                    nc.tensor.matmul(
                        p[:, 0:256],
                        xt_s[(kc, nb)][:, 128 * tq:128 * (tq + 1)],
                        wv_s[kc][:], start=(kc == 0), stop=(kc == 7))
                tci = 4 * nb + tq
                for h in range(HPC):
                    nc.vector.tensor_copy(
                        v_s[h][:, 65 * tci:65 * tci + 64],
                        p[:, 64 * h:64 * (h + 1)])
            return run

        def qkv_chunks(nb):
            return ([qk_chunk(nb, mc) for mc in range(4)]
                    + [v_chunk(nb, tq) for tq in range(4)])

        with tc.tile_pool(name="pb", bufs=1) as pb:
            filler = deque()

            def pop_filler():
                if filler:
                    filler.popleft()()

            def make_normproj(ib):
                def fin():
                    recs = pb.tile([4, 512], F32R, tag="recs", bufs=2,
                                   name="recs")
                    # f32r shares f32 bits; TF32 rounding only on PE read
                    with nc.allow_low_precision(reason="f32r == f32 bits"):
                        nc.vector.reciprocal(recs[:], drows[ib][:])
                    for h in range(HPC):
                        pr, sub = h // 2, h % 2
                        prf = psb.tile([64, 512], F32, tag="bcast")
                        nc.tensor.matmul(prf[:],
                                         sel_s[:, 64 * h:64 * (h + 1)],
                                         recs[:], start=True, stop=True)
                        if sub == 0:
                            nc.vector.tensor_mul(
                                yT[pr][0:64, 512 * ib:512 * (ib + 1)],
                                yT[pr][0:64, 512 * ib:512 * (ib + 1)],
                                prf[:])
                        else:
                            oidx = 4 * pr + ib
                            blk = ytmp[:, 512 * oidx:512 * (oidx + 1)]
                            nc.vector.tensor_mul(blk, blk, prf[:])
                            nc.scalar.dma_start(
                                yT[pr][64:128, 512 * ib:512 * (ib + 1)],
                                blk)
                    for tb in range(4 * ib, 4 * ib + 4):
                        for n in range(2):
                            p = psm.tile([128, 512], F32, tag="mm")
                            for pp in range(2):
                                nc.tensor.matmul(
                                    p[:],
                                    yT[pp][:, 128 * tb:128 * (tb + 1)],
                                    wp_s[pp][:, 512 * n:512 * (n + 1)],
                                    start=(pp == 0), stop=(pp == 1))
                            o = pb.tile([128, 512], BF16, tag="o", bufs=2,
                                        name="o")
                            nc.vector.tensor_copy(o[:], p[:])
                            nc.sync.dma_start(
                                out_t.ap()[128 * tb:128 * (tb + 1),
                                           512 * n:512 * (n + 1)], o[:])
                return fin

            # nb=0 qkv runs dense up front (nothing to interleave with)
            for chunk in qkv_chunks(0):
                chunk()

            pending = None
            for ib in range(TB):
                # filler supply: qkv(ib+1); qkv(3)'s v-chunks are held
                # back for ib=3 (needed only by its diagonal AV, late)
                if ib + 1 < TB:
                    if ib + 1 < 3:
                        filler.extend(qkv_chunks(ib + 1))
                    else:
                        filler.extend(qk_chunk(3, mc) for mc in range(4))
                else:
                    filler.extend(v_chunk(3, tq) for tq in range(4))
                nblk = HPC * (4 * ib + 4)
                stride = max(1, nblk // max(len(filler), 1))
                if ib == TB - 1:
                    stride = 2  # drain v(3) early, within h=0
                blk_i = 0
                for h in range(HPC):
                    pr, sub = h // 2, h % 2
                    qTr = qT[pr][64 * sub:64 * (sub + 1), :]
                    kTr = kT[pr][64 * sub:64 * (sub + 1), :]
                    jhi = 4 * ib + 3
                    i0 = 512 * ib
                    py = psy.tile([65, 512], F32, tag="avy")
                    avq = []

                    def emit_av(ent, py=py, h=h, jhi=jhi):
                        jc, mov, off = ent
                        nc.tensor.matmul(
                            py[:, off:512],
                            v_s[h][:, 65 * jc:65 * jc + 65],
                            mov, start=(jc == 0), stop=(jc == jhi))

                    for jc in range(jhi + 1):
                        r = jc - 4 * ib
                        off = 128 * r if r > 0 else 0
                        w = 512 - off
                        p = psm.tile([128, 512], F32, tag="mm")
                        nc.tensor.matmul(
                            p[:, 0:w],
                            kTr[:, 128 * jc:128 * (jc + 1)],
                            qTr[:, i0 + off:i0 + 512],
                            start=True, stop=True)
                        pt = pb.tile([128, 512], BF16, tag="P", bufs=4,
                                     name="pt")
                        nc.scalar.activation(pt[:, 0:w], p[:, 0:w], Exp,
                                             scale=0.125)
                        if r >= 0:
                            nc.gpsimd.tensor_mul(
                                pt[:, 0:128], pt[:, 0:128], trim_s[:])
                        avq.append((jc, pt[:, 0:w], off))
                        blk_i += 1
                        if blk_i % stride == 0:
                            pop_filler()
                        if jc == 6 and pending is not None:
                            pending()
                            pending = None
                        if len(avq) > 2:
                            emit_av(avq.pop(0))
                    while avq:
                        emit_av(avq.pop(0))
                    # denominator row + unnormalized y staging
                    dtmp = pb.tile([1, 512], F32, tag="dtmp", bufs=2)
                    nc.vector.tensor_copy(dtmp[:], py[64:65, :])
                    nc.scalar.dma_start(drows[ib][h:h + 1, :], dtmp[:])
                    if sub == 0:
                        nc.vector.tensor_copy(
                            yT[pr][0:64, i0:i0 + 512], py[0:64, :])
                    else:
                        oidx = 4 * pr + ib
                        nc.vector.tensor_copy(
                            ytmp[:, 512 * oidx:512 * (oidx + 1)],
                            py[0:64, :])
                # all of qkv(ib+1) must precede attention(ib+1)
                while filler:
                    pop_filler()
                pending = make_normproj(ib)
            pending()

    nc.compile()
    return nc


def _get_compiled():
    global _compiled
    if _compiled is None:
        _compiled = _build_nc()
    return _compiled


def _in_maps(x, w_qkv, w_proj):
    x = np.asarray(x, dtype=np.float32)
    w_qkv = np.asarray(w_qkv, dtype=np.float32)
    w_proj = np.asarray(w_proj, dtype=np.float32)
    trim = _build_trim()
    sel = np.zeros((4, 4 * 64), dtype=np.float32)
    for b in range(4):
        sel[b, 64 * b:64 * (b + 1)] = 1.0
    maps = []
    for core in range(N_CORES):
        b = core // 4
        h0 = 4 * (core % 4)
        heads = range(h0, h0 + HPC)
        xt = np.ascontiguousarray(
            x[b].T.astype(NPBF).reshape(8, 128, T).transpose(1, 0, 2)
            .reshape(128, 8 * T))
        wq = np.concatenate(
            [w_qkv[:, 64 * h:64 * (h + 1)] for h in heads], axis=1)
        wk = np.concatenate(
            [w_qkv[:, C + 64 * h:C + 64 * (h + 1)] for h in heads], axis=1)
        wqk = np.concatenate([wq, wk], axis=1).astype(NPBF)  # [C, 512]
        wqk = np.ascontiguousarray(
            wqk.reshape(8, 128, 512).transpose(1, 0, 2).reshape(128, -1))
        wv = np.concatenate(
            [w_qkv[:, 2 * C + 64 * h:2 * C + 64 * (h + 1)] for h in heads],
            axis=1).astype(NPBF)  # [C, 256]
        wv = np.ascontiguousarray(
            wv.reshape(8, 128, 256).transpose(1, 0, 2).reshape(128, -1))
        wp = np.concatenate(
            [w_proj[64 * h:64 * (h + 1), :] for h in heads],
            axis=0).astype(NPBF)  # [256, C]
        wp = np.ascontiguousarray(
            wp.reshape(2, 128, C).transpose(1, 0, 2).reshape(128, 2 * C))
        maps.append({
            "xt": xt,
            "wqk": wqk,
            "wv": wv,
            "wp": wp,
            "trim": trim,
            "sel": sel,
        })
    return maps


def _combine(results, b_proj):
    out = np.zeros((B, T, C), dtype=np.float32)
    for core in range(N_CORES):
        out[core // 4] += np.asarray(results[core]["out"],
                                     dtype=np.float32)
    out += np.asarray(b_proj, dtype=np.float32)[None, None, :]
    return out


def kernel(x, w_qkv, w_proj, b_proj):
    nc = _get_compiled()
    res = run_bass_kernel_spmd(nc, _in_maps(x, w_qkv, w_proj),
                               core_ids=list(range(N_CORES)))
    return _combine(res.results, b_proj)


def kernel_traced(x, w_qkv, w_proj, b_proj):
    """Like kernel() but with NTFF tracing; returns (out, results)."""
    nc = _get_compiled()
    res = run_bass_kernel_spmd(nc, _in_maps(x, w_qkv, w_proj),
                               core_ids=list(range(N_CORES)), trace=True)
    return _combine(res.results, b_proj), res
